# revision 5
# baseline (speedup 1.0000x reference)
"""DINO loss kernel for Trainium2 (8 NeuronCores, Bass/Tile) — v2.

Math: with S = student.reshape(640, D), T = teacher.reshape(128, D),
P = softmax((T - center)/tau), L = log_softmax(S/0.1), M = P @ L.T,
loss = -(sum(M) - trace(M)) / (128*639).

Decomposition (s = 10*S, c_v = logsumexp_d(s[v])):
  sum(M)   = dot(colsum_P, colsum_s) - 128*C        C   = sum_v c_v
  trace(M) = TR - C128                              TR  = sum_i dot(P_i, s_i)

Row sharding: core k owns teacher rows [16k,16k+16), student rows
[16k,16k+16) ("trace" block) and [128+64k, 128+64k+64) ("main" block).
Teacher/trace blocks pack into 128 partitions as (row, 8 segs of 8192);
main packs as (row, 2 segs of 32768). All inputs are bf16 (tolerance is
2e-2; bf16 costs ~1e-4).

Per-core engine assignment:
  ACT    exp of teacher (shift 40) + trace + most main columns (shift 15),
         row-sum partials via fused accum_out (fp32).
  DVE    fast-exp (2^x bit trick: y*A+B -> int32 -> bitcast f32) on a slice
         of the main columns, tensor_tensor_reduce for the trace term,
         PSUM evictions, zoo builds. Fast-exp sums carry a known +4.07%
         chord bias, corrected on the host.
  PE     column sums as matmuls with stationary zero-padded [128,128]
         selector weights, 64 (or 16) matmuls accumulating into one
         [128,512] PSUM bank. Teacher selectors carry 1/Z so the softmax
         scaling is fused into the reduction.
Scalars/column-sums are combined on the host in f64.

Measured anatomy at ~57-59us exec (trace-verified): ~7us NEFF preamble,
~37us bf16 input stream (near HBM peak, fully overlapped), ACT exp chain
ends ~51us (critical path; 1 elem/lane/cycle @1.2GHz is a hard floor),
DVE co-finishes ~50us, then a ~7us tail: ~4us HWDGE issue+descriptor
latency for the final output DMA, ~4us HBM-write completion-semaphore
latency (doc'd WAW delay), ~2.5us postamble barriers. Next levers (each
~2-3us, at jitter level): per-chunk psumM banks (M0..3, reuse psumT for
warm-ups) to evict/flush stageM incrementally; fp8 for exp-only main
columns. Do NOT: gpsimd elementwise (starves DVE via shared SBUF port),
tensor_tensor_reduce (hangs HW), gpsimd tensor_scalar (ncc rejects),
untagged pool tiles (alias one slot), in-place accum ops (drop to 1x).
"""

import dataclasses
import numpy as np

D = 65536
NCORES = 8
FB = 8192                  # big free-dim chunk (per-partition columns)
KT = 40.0                  # teacher exp shift: exp(t - KT), cancels in softmax
KS = 15.0                  # student exp shift: logsumexp = KS + log(sum exp(s-KS))

LOG2E = float(np.log2(np.e))
FE_A = 10.0 * LOG2E * (1 << 23)            # fast-exp scale (s = 10*x)
FE_B = (127.0 - KS * LOG2E) * (1 << 23)    # fast-exp bias
FE_CORR = 1.0406620487668925               # E[(1+f)/2^f], f~U[0,1): chord bias

# Per main chunk i (4 chunks of 8192): DVE fast-exp takes the last DVE_W[i]
# columns, ACT the rest.
DVE_W = (3584, 3584, 3584, 0)
N_WARM = 32                # PE warm-up matmuls (HAM clock-gate)

_CACHE = {}

TRACE = False              # test harness sets kernel.TRACE = True for profiling
LAST_RESULTS = None        # stashed BassKernelResults for the test harness

# OUT tile column map (fp32 [128, 1552])
O_CT = 0                   # 0:512    teacher colsum_P partials
O_CR = 512                 # 512:1024 trace raw colsum partials
O_CM = 1024                # 1024:1536 main raw colsum partials
O_ZT = 1536                # 1536:1540 teacher row-sum (Z) partials (4 sub)
O_SL = 1540                # trace logsumexp partials
O_TR = 1541                # trace-term partials sum_d E*s (DVE half)
O_TR2 = 1542               # trace-term partials (ACT half)
O_SMA = 1543               # 1543:1547 main ACT logsumexp partials (chunk i)
O_SMD = 1547               # 1547:1551 main DVE fast-exp partials (chunk i)
O_Z1 = 1551                # teacher Z partial total (sum of the 4 sub cols)
O_SM3 = 1552               # 1552:1556 tail-chunk sub-accums (4 x 2048 cols)
O_W = 1556


def _strided(ap, dims):
    """Replace the free dims of a 2D AP with explicit [step, num] dims."""
    return dataclasses.replace(ap, ap=[ap.ap[0]] + [list(d) for d in dims])


def _build_program():
    import concourse.tile as tile
    from concourse import bacc
    from concourse import mybir

    fp32 = mybir.dt.float32
    bf16 = mybir.dt.bfloat16
    i32 = mybir.dt.int32
    Exp = mybir.ActivationFunctionType.Exp
    AX = mybir.AxisListType.X
    MUL = mybir.AluOpType.mult
    ADD = mybir.AluOpType.add

    nc = bacc.Bacc(None, target_bir_lowering=False)

    xt = nc.dram_tensor("xt", [128, FB], bf16, kind="ExternalInput")
    xst = nc.dram_tensor("xst", [128, FB], bf16, kind="ExternalInput")
    xsm = nc.dram_tensor("xsm", [128, 4 * FB], bf16, kind="ExternalInput")
    par8 = nc.dram_tensor("par8", [128, 8], bf16, kind="ExternalInput")
    par2 = nc.dram_tensor("par2", [128, 2], bf16, kind="ExternalInput")
    b16 = nc.dram_tensor("b16", [128, 128], fp32, kind="ExternalInput")

    o_out = nc.dram_tensor("out", [128, O_W], fp32, kind="ExternalOutput")

    with tile.TileContext(nc) as tc:
        with (
            tc.tile_pool(name="singles", bufs=1) as singles,
            tc.tile_pool(name="psum", bufs=1, space="PSUM") as psum,
        ):
            # ---- input loads (issued up-front; HW drains them in order) ----
            # teacher + trace interleaved in 2048-col sub-loads so ACT can
            # start early; mains whole except the last, split for the tail
            xt_t = singles.tile([128, FB], bf16, tag="xt_t")
            xst_t = singles.tile([128, FB], bf16, tag="xst_t")
            p8 = singles.tile([128, 8], bf16, tag="p8")
            p2 = singles.tile([128, 2], bf16, tag="p2")
            b16t = singles.tile([128, 128], fp32, tag="b16t")
            for j in range(4):
                sl = slice(2048 * j, 2048 * (j + 1))
                nc.sync.dma_start(out=xt_t[:, sl], in_=xt[:, sl])
                nc.sync.dma_start(out=xst_t[:, sl], in_=xst[:, sl])
                if j == 0:
                    nc.sync.dma_start(out=p8, in_=par8[:, :])
                    nc.sync.dma_start(out=p2, in_=par2[:, :])
                    nc.sync.dma_start(out=b16t, in_=b16[:, :])
            xsm_t = []
            for i in range(4):
                t = singles.tile([128, FB], bf16, name=f"xsm{i}",
                                 tag=f"xsm{i}")
                if i < 3:
                    nc.sync.dma_start(out=t, in_=xsm[:, i * FB:(i + 1) * FB])
                else:
                    for j in range(4):
                        sl = slice(2048 * j, 2048 * (j + 1))
                        nc.sync.dma_start(
                            out=t[:, sl], in_=xsm[:, i * FB:][:, sl])
                xsm_t.append(t)

            # ---- residents / scratch ----
            OUT = singles.tile([128, O_W], fp32, tag="OUT")
            E = singles.tile([128, FB], bf16, tag="E")
            zoo2 = singles.tile([128, 64 * 128], bf16, tag="zoo2")
            zoo8 = singles.tile([128, 16 * 128], bf16, tag="zoo8")
            zooT = singles.tile([128, 16 * 128], bf16, tag="zooT")
            warm_t = singles.tile([128, 512], bf16, tag="warm_t")
            sc_act = singles.tile([128, FB], bf16, tag="sc_act")
            sc_ttr = singles.tile([128, FB], bf16, tag="sc_ttr")
            sc_i32 = singles.tile([128, 4096], i32, tag="sc_i32")
            rexp8 = singles.tile([128, 8], bf16, tag="rexp8")
            rb = singles.tile([128, 1], fp32, tag="rb")
            bias_t = singles.tile([128, 1], fp32, tag="bias_t")
            bias_s = singles.tile([128, 1], fp32, tag="bias_s")

            psumT = psum.tile([128, 512], fp32, tag="psumT")
            psumR = psum.tile([128, 512], fp32, tag="psumR")
            psumMs = [psum.tile([128, 512], fp32, name=f"psumM{i}",
                                tag=f"psumM{i}")
                      for i in range(4)]
            psumW = psum.tile([128, 512], fp32, tag="psumW")
            psumZt = psum.tile([128, 512], fp32, tag="psumZt")  # full bank
            psumZ = psumZt[:, 0:1]

            # ---- constant setup (DVE memsets tiny; zoo zeroing on GPSIMD) ----
            nc.vector.memset(bias_t, -KT)
            nc.vector.memset(bias_s, -KS)
            nc.vector.memset(warm_t, 0.0)
            nc.vector.memset(OUT[:, O_ZT:O_W], 0.0)
            # prefetch the exp table set while DMA streams in
            nc.scalar.activation(out=rb, in_=bias_s, func=Exp, bias=bias_s,
                                 scale=1.0)

            def zero_fill(dst_tile, nrep):
                nc.vector.tensor_copy(
                    _strided(dst_tile[:, :], [[512, nrep], [1, 512]]),
                    _strided(warm_t[:, :], [[0, nrep], [1, 512]]))

            zero_fill(zoo8, 4)
            zero_fill(zooT, 4)
            zero_fill(zoo2, 16)

            # PE warm-up: keep the HAM clock-gate open until real work lands
            for w in range(N_WARM):
                nc.tensor.matmul(psumW, warm_t[:, 0:128], warm_t[:, 0:512],
                                 start=True, stop=True)

            nc.vector.tensor_copy(
                _strided(zoo8[:, :], [[136, 16], [1, 8]]),
                _strided(p8[:, :], [[0, 16], [1, 8]]))
            nc.vector.tensor_copy(
                _strided(zoo2[:, :], [[130, 64], [1, 2]]),
                _strided(p2[:, :], [[0, 64], [1, 2]]))

            # ---- teacher: E = exp(10*xt - 40) in 4 sub-chunks ----
            for j in range(4):
                sl = slice(2048 * j, 2048 * (j + 1))
                nc.scalar.activation(
                    out=E[:, sl], in_=xt_t[:, sl], func=Exp, bias=bias_t,
                    scale=10.0, accum_out=OUT[:, O_ZT + j:O_ZT + j + 1])

            # ---- trace colsums on PE (ready before teacher weights) ----
            for c in range(16):
                nc.tensor.matmul(psumR, zoo8[:, 128 * c:128 * (c + 1)],
                                 xst_t[:, 512 * c:512 * (c + 1)],
                                 start=(c == 0), stop=(c == 15))

            # ---- teacher Z -> 1/Z -> weighted selectors ----
            nc.vector.tensor_scalar(
                out=OUT[:, O_ZT:O_ZT + 4], in0=OUT[:, O_ZT:O_ZT + 4],
                scalar1=1.0, scalar2=None, op0=MUL, op1=ADD,
                accum_out=OUT[:, O_Z1:O_Z1 + 1])
            nc.tensor.matmul(psumZ, b16t, OUT[:, O_Z1:O_Z1 + 1],
                             start=True, stop=True)
            nc.vector.reciprocal(out=rb, in_=psumZ)
            nc.vector.tensor_scalar_mul(out=rexp8, in0=p8, scalar1=rb)
            nc.vector.tensor_copy(
                _strided(zooT[:, :], [[136, 16], [1, 8]]),
                _strided(rexp8[:, :], [[0, 16], [1, 8]]))
            for c in range(16):
                nc.tensor.matmul(psumT, zooT[:, 128 * c:128 * (c + 1)],
                                 E[:, 512 * c:512 * (c + 1)],
                                 start=(c == 0), stop=(c == 15))

            # ---- trace block: exp (ACT), E*s term split DVE/ACT ----
            nc.scalar.activation(out=sc_act, in_=xst_t, func=Exp, bias=bias_s,
                                 scale=10.0, accum_out=OUT[:, O_SL:O_SL + 1])
            # tensor_tensor_reduce hangs trn2 hw; mul + fused-accum instead.
            # Two halves: half A starts as soon as E[0:4096] exists.
            nc.vector.tensor_mul(sc_ttr[:, 0:4096], E[:, 0:4096],
                                 xst_t[:, 0:4096])
            nc.vector.tensor_scalar(
                out=sc_ttr[:, 0:4096], in0=sc_ttr[:, 0:4096],
                scalar1=1.0, scalar2=None, op0=MUL, op1=ADD,
                accum_out=OUT[:, O_TR:O_TR + 1])
            nc.vector.tensor_mul(sc_ttr[:, 4096:8192], E[:, 4096:8192],
                                 xst_t[:, 4096:8192])
            nc.vector.tensor_copy(OUT[:, O_CR:O_CR + 512], psumR)
            # stream the trace colsums out as soon as they exist: the final
            # DMA's HWDGE issue+descriptor latency otherwise sits in the tail
            nc.sync.dma_start(out=o_out[:, O_CR:O_CR + 512],
                              in_=OUT[:, O_CR:O_CR + 512])

            # ---- main blocks ----
            Copy = mybir.ActivationFunctionType.Copy
            for i in range(4):
                w = DVE_W[i]
                aw = FB - w
                if i < 3:
                    nc.scalar.activation(
                        out=sc_act[:, 0:aw], in_=xsm_t[i][:, 0:aw], func=Exp,
                        bias=bias_s, scale=10.0,
                        accum_out=OUT[:, O_SMA + i:O_SMA + i + 1])
                else:
                    # tail chunk: sub-split so compute tracks the sub-loads
                    for j in range(4):
                        sl = slice(2048 * j, 2048 * (j + 1))
                        nc.scalar.activation(
                            out=sc_act[:, sl], in_=xsm_t[i][:, sl], func=Exp,
                            bias=bias_s, scale=10.0,
                            accum_out=OUT[:, O_SM3 + j:O_SM3 + j + 1])
                if w > 0:
                    nc.vector.tensor_scalar(
                        out=sc_i32[:, 0:w], in0=xsm_t[i][:, aw:FB],
                        scalar1=FE_A, scalar2=FE_B, op0=MUL, op1=ADD)
                    fview = sc_i32[:, 0:w].bitcast(fp32)
                    nc.vector.tensor_scalar(
                        out=fview, in0=fview, scalar1=1.0, scalar2=None,
                        op0=MUL, op1=ADD,
                        accum_out=OUT[:, O_SMD + i:O_SMD + i + 1])
                # per-chunk psum bank: chunk i only writes partitions
                # [32i, 32i+32), so each bank closes after 16 matmuls and
                # its colsum stripe evicts + streams out immediately
                for c in range(16):
                    k = 16 * i + c
                    nc.tensor.matmul(psumMs[i],
                                     zoo2[:, 128 * k:128 * (k + 1)],
                                     xsm_t[i][:, 512 * c:512 * (c + 1)],
                                     start=(c == 0), stop=(c == 15))
                st = slice(32 * i, 32 * (i + 1))
                nc.vector.tensor_copy(OUT[st, O_CM:O_CM + 512],
                                      psumMs[i][st, :])
                nc.sync.dma_start(out=o_out[st, O_CM:O_CM + 512],
                                  in_=OUT[st, O_CM:O_CM + 512])
                if i == 0:
                    nc.vector.tensor_copy(OUT[:, O_CT:O_CT + 512], psumT)
                    nc.sync.dma_start(out=o_out[:, O_CT:O_CT + 512],
                                      in_=OUT[:, O_CT:O_CT + 512])
                    # second product half reduced on DVE
                    nc.vector.tensor_scalar(
                        out=sc_ttr[:, 4096:8192], in0=sc_ttr[:, 4096:8192],
                        scalar1=1.0, scalar2=None, op0=MUL, op1=ADD,
                        accum_out=OUT[:, O_TR2:O_TR2 + 1])

            # ---- final write: only the scalar accumulator columns ----
            nc.sync.dma_start(out=o_out[:, 1536:O_W], in_=OUT[:, 1536:O_W])

    nc.compile()
    return nc


def _get_program():
    key = ("nc_v2", DVE_W)
    if key not in _CACHE:
        _CACHE[key] = _build_program()
    return _CACHE[key]


def _host_tensors():
    par8 = np.tile(np.eye(8, dtype=np.float32), (16, 1))
    par2 = np.tile(np.eye(2, dtype=np.float32), (64, 1))
    b16 = np.kron(np.eye(16, dtype=np.float32), np.ones((8, 8), np.float32))
    return par8, par2, b16


def _make_in_maps(student_output, teacher_output, center, epoch):
    import ml_dtypes

    bf = ml_dtypes.bfloat16
    S = np.asarray(student_output, dtype=np.float32).reshape(-1, D)   # [640, D]
    T = np.asarray(teacher_output, dtype=np.float32).reshape(-1, D)   # [128, D]
    cen = np.asarray(center, dtype=np.float32).reshape(1, D)
    ep = int(np.asarray(epoch))
    if ep < 30:
        t_temp = 0.04 + (0.07 - 0.04) * ep / 30
    else:
        t_temp = 0.07

    # fold center + temperature so the device uses one scale (10)
    tpre = ((T - cen) * np.float32(1.0 / (t_temp * 10.0))).astype(bf)
    Sb = S.astype(bf)

    par8, par2, b16 = _host_tensors()
    par8 = par8.astype(bf)
    par2 = par2.astype(bf)
    in_maps = []
    for k in range(NCORES):
        in_maps.append({
            "xt": np.ascontiguousarray(
                tpre[16 * k:16 * (k + 1)].reshape(128, FB)),
            "xst": np.ascontiguousarray(
                Sb[16 * k:16 * (k + 1)].reshape(128, FB)),
            "xsm": np.ascontiguousarray(
                Sb[128 + 64 * k:128 + 64 * (k + 1)].reshape(128, 4 * FB)),
            "par8": par8, "par2": par2, "b16": b16,
        })
    return in_maps


def _combine(outs):
    """outs: list of 8 [128, O_W] f32 arrays -> final loss (f64 host math)."""
    colP = np.zeros(D)
    colS = np.zeros(D)
    C = 0.0
    C128 = 0.0
    TR = 0.0
    for k in range(NCORES):
        o = outs[k].astype(np.float64)
        # teacher colsum_P: [8c+e, j] -> d = e*8192 + c*512 + j
        colP += o[:, O_CT:O_CT + 512].reshape(16, 8, 512) \
            .transpose(1, 0, 2).reshape(-1)
        colS += o[:, O_CR:O_CR + 512].reshape(16, 8, 512) \
            .transpose(1, 0, 2).reshape(-1)
        # main colsum: [2k+s, j] -> d = s*32768 + k*512 + j
        colS += o[:, O_CM:O_CM + 512].reshape(64, 2, 512) \
            .transpose(1, 0, 2).reshape(-1)

        z = o[:, O_ZT:O_ZT + 4].sum(axis=1).reshape(16, 8).sum(axis=1)
        zs_tr = o[:, O_SL].reshape(16, 8).sum(axis=1)
        c_tr = KS + np.log(zs_tr)
        zs_m = o[:, O_SMA:O_SMA + 4].copy()
        for i in range(4):
            if DVE_W[i] > 0:
                zs_m[:, i] += o[:, O_SMD + i] / FE_CORR
            if DVE_W[i] == FB:
                zs_m[:, i] = o[:, O_SMD + i] / FE_CORR
        zs_rows = (zs_m.sum(axis=1) + o[:, O_SM3:O_SM3 + 4].sum(axis=1)) \
            .reshape(64, 2).sum(axis=1)
        c_m = KS + np.log(zs_rows)
        tr_rows = (o[:, O_TR] + o[:, O_TR2]).reshape(16, 8).sum(axis=1)

        C += c_tr.sum() + c_m.sum()
        C128 += c_tr.sum()
        TR += (10.0 * tr_rows / z).sum()

    s_pl = colP @ (10.0 * colS)
    total = s_pl - 128.0 * C - TR + C128
    loss = -total / (128.0 * 639.0)
    return np.array(loss, dtype=np.float32)


def kernel(student_output, teacher_output, center, epoch):
    from concourse.bass_utils import run_bass_kernel_spmd

    global LAST_RESULTS

    in_maps = _make_in_maps(student_output, teacher_output, center, epoch)
    nc = _get_program()
    res = run_bass_kernel_spmd(
        nc, in_maps, core_ids=list(range(NCORES)), trace=TRACE)
    LAST_RESULTS = res
    return _combine([res.results[k]["out"] for k in range(NCORES)])



# revision 6
# speedup vs baseline: 1.0206x; 1.0206x over previous
"""DINO loss kernel for Trainium2 (8 NeuronCores, Bass/Tile) — v2.

Math: with S = student.reshape(640, D), T = teacher.reshape(128, D),
P = softmax((T - center)/tau), L = log_softmax(S/0.1), M = P @ L.T,
loss = -(sum(M) - trace(M)) / (128*639).

Decomposition (s = 10*S, c_v = logsumexp_d(s[v])):
  sum(M)   = dot(colsum_P, colsum_s) - 128*C        C   = sum_v c_v
  trace(M) = TR - C128                              TR  = sum_i dot(P_i, s_i)

Row sharding: core k owns teacher rows [16k,16k+16), student rows
[16k,16k+16) ("trace" block) and [128+64k, 128+64k+64) ("main" block).
Teacher/trace blocks pack into 128 partitions as (row, 8 segs of 8192);
main packs as (row, 2 segs of 32768). All inputs are bf16 (tolerance is
2e-2; bf16 costs ~1e-4).

Per-core engine assignment:
  ACT    exp of teacher (shift 40) + trace + most main columns (shift 15),
         row-sum partials via fused accum_out (fp32).
  DVE    fast-exp (2^x bit trick: y*A+B -> int32 -> bitcast f32) on a slice
         of the main columns, tensor_tensor_reduce for the trace term,
         PSUM evictions, zoo builds. Fast-exp sums carry a known +4.07%
         chord bias, corrected on the host.
  PE     column sums as matmuls with stationary zero-padded [128,128]
         selector weights, 64 (or 16) matmuls accumulating into one
         [128,512] PSUM bank. Teacher selectors carry 1/Z so the softmax
         scaling is fused into the reduction.
Scalars/column-sums are combined on the host in f64.

Measured anatomy at ~57-59us exec (trace-verified): ~7us NEFF preamble,
~37us bf16 input stream (near HBM peak, fully overlapped), ACT exp chain
ends ~51us (critical path; 1 elem/lane/cycle @1.2GHz is a hard floor),
DVE co-finishes ~50us, then a ~7us tail: ~4us HWDGE issue+descriptor
latency for the final output DMA, ~4us HBM-write completion-semaphore
latency (doc'd WAW delay), ~2.5us postamble barriers. Next levers (each
~2-3us, at jitter level): per-chunk psumM banks (M0..3, reuse psumT for
warm-ups) to evict/flush stageM incrementally; fp8 for exp-only main
columns. Do NOT: gpsimd elementwise (starves DVE via shared SBUF port),
tensor_tensor_reduce (hangs HW), gpsimd tensor_scalar (ncc rejects),
untagged pool tiles (alias one slot), in-place accum ops (drop to 1x).
"""

import dataclasses
import numpy as np

D = 65536
NCORES = 8
FB = 8192                  # big free-dim chunk (per-partition columns)
KT = 40.0                  # teacher exp shift: exp(t - KT), cancels in softmax
KS = 15.0                  # student exp shift: logsumexp = KS + log(sum exp(s-KS))

LOG2E = float(np.log2(np.e))
FE_A = 10.0 * LOG2E * (1 << 23)            # fast-exp scale (s = 10*x)
FE_B = (127.0 - KS * LOG2E) * (1 << 23)    # fast-exp bias
FE_CORR = 1.0406620487668925               # E[(1+f)/2^f], f~U[0,1): chord bias

# Per main chunk i (4 chunks of 8192): DVE fast-exp takes the last DVE_W[i]
# columns, ACT the rest.
DVE_W = (3584, 3584, 3584, 0)
N_WARM = 32                # PE warm-up matmuls (HAM clock-gate)

_CACHE = {}

TRACE = False              # test harness sets kernel.TRACE = True for profiling
LAST_RESULTS = None        # stashed BassKernelResults for the test harness

# OUT tile column map (fp32 [128, 1552])
O_CT = 0                   # 0:512    teacher colsum_P partials
O_CR = 512                 # 512:1024 trace raw colsum partials
O_CM = 1024                # 1024:1536 main raw colsum partials
O_ZT = 1536                # 1536:1540 teacher row-sum (Z) partials (4 sub)
O_SL = 1540                # trace logsumexp partials
O_TR = 1541                # trace-term partials sum_d E*s (DVE half)
O_TR2 = 1542               # trace-term partials (ACT half)
O_SMA = 1543               # 1543:1547 main ACT logsumexp partials (chunk i)
O_SMD = 1547               # 1547:1551 main DVE fast-exp partials (chunk i)
O_Z1 = 1551                # teacher Z partial total (sum of the 4 sub cols)
O_SM3 = 1552               # 1552:1556 tail-chunk sub-accums (4 x 2048 cols)
O_W = 1556


def _strided(ap, dims):
    """Replace the free dims of a 2D AP with explicit [step, num] dims."""
    return dataclasses.replace(ap, ap=[ap.ap[0]] + [list(d) for d in dims])


def _build_program():
    import concourse.tile as tile
    from concourse import bacc
    from concourse import mybir

    fp32 = mybir.dt.float32
    bf16 = mybir.dt.bfloat16
    i32 = mybir.dt.int32
    Exp = mybir.ActivationFunctionType.Exp
    AX = mybir.AxisListType.X
    MUL = mybir.AluOpType.mult
    ADD = mybir.AluOpType.add

    nc = bacc.Bacc(None, target_bir_lowering=False)

    xt = nc.dram_tensor("xt", [128, FB], bf16, kind="ExternalInput")
    xst = nc.dram_tensor("xst", [128, FB], bf16, kind="ExternalInput")
    xsm = nc.dram_tensor("xsm", [128, 4 * FB], bf16, kind="ExternalInput")
    par8 = nc.dram_tensor("par8", [128, 8], bf16, kind="ExternalInput")
    par2 = nc.dram_tensor("par2", [128, 2], bf16, kind="ExternalInput")
    b16 = nc.dram_tensor("b16", [128, 128], fp32, kind="ExternalInput")

    o_out = nc.dram_tensor("out", [128, O_W], fp32, kind="ExternalOutput")

    with tile.TileContext(nc) as tc:
        with (
            tc.tile_pool(name="singles", bufs=1) as singles,
            tc.tile_pool(name="psum", bufs=1, space="PSUM") as psum,
        ):
            # ---- input loads (issued up-front; HW drains them in order) ----
            # teacher + trace interleaved in 2048-col sub-loads so ACT can
            # start early; mains whole except the last, split for the tail
            xt_t = singles.tile([128, FB], bf16, tag="xt_t")
            xst_t = singles.tile([128, FB], bf16, tag="xst_t")
            p8 = singles.tile([128, 8], bf16, tag="p8")
            p2 = singles.tile([128, 2], bf16, tag="p2")
            b16t = singles.tile([128, 128], fp32, tag="b16t")
            for j in range(4):
                sl = slice(2048 * j, 2048 * (j + 1))
                nc.sync.dma_start(out=xt_t[:, sl], in_=xt[:, sl])
                nc.sync.dma_start(out=xst_t[:, sl], in_=xst[:, sl])
                if j == 0:
                    nc.sync.dma_start(out=p8, in_=par8[:, :])
                    nc.sync.dma_start(out=p2, in_=par2[:, :])
                    nc.sync.dma_start(out=b16t, in_=b16[:, :])
            xsm_t = []
            for i in range(4):
                t = singles.tile([128, FB], bf16, name=f"xsm{i}",
                                 tag=f"xsm{i}")
                if i < 3:
                    nc.sync.dma_start(out=t, in_=xsm[:, i * FB:(i + 1) * FB])
                else:
                    for j in range(4):
                        sl = slice(2048 * j, 2048 * (j + 1))
                        nc.sync.dma_start(
                            out=t[:, sl], in_=xsm[:, i * FB:][:, sl])
                xsm_t.append(t)

            # ---- residents / scratch ----
            OUT = singles.tile([128, O_W], fp32, tag="OUT")
            E = singles.tile([128, FB], bf16, tag="E")
            zoo2 = singles.tile([128, 64 * 128], bf16, tag="zoo2")
            zoo8 = singles.tile([128, 16 * 128], bf16, tag="zoo8")
            zooT = singles.tile([128, 16 * 128], bf16, tag="zooT")
            warm_t = singles.tile([128, 512], bf16, tag="warm_t")
            sc_act = singles.tile([128, FB], bf16, tag="sc_act")
            sc_ttr = singles.tile([128, FB], bf16, tag="sc_ttr")
            sc_i32 = singles.tile([128, 4096], i32, tag="sc_i32")
            rexp8 = singles.tile([128, 8], bf16, tag="rexp8")
            rb = singles.tile([128, 1], fp32, tag="rb")
            bias_t = singles.tile([128, 1], fp32, tag="bias_t")
            bias_s = singles.tile([128, 1], fp32, tag="bias_s")

            psumT = psum.tile([128, 512], fp32, tag="psumT")
            psumR = psum.tile([128, 512], fp32, tag="psumR")
            psumM = psum.tile([128, 512], fp32, tag="psumM")
            psumW = psum.tile([128, 512], fp32, tag="psumW")
            psumZt = psum.tile([128, 512], fp32, tag="psumZt")  # full bank
            psumZ = psumZt[:, 0:1]

            # ---- constant setup (DVE memsets tiny; zoo zeroing on GPSIMD) ----
            nc.vector.memset(bias_t, -KT)
            nc.vector.memset(bias_s, -KS)
            nc.vector.memset(warm_t, 0.0)
            nc.vector.memset(OUT[:, O_ZT:O_W], 0.0)
            # prefetch the exp table set while DMA streams in
            nc.scalar.activation(out=rb, in_=bias_s, func=Exp, bias=bias_s,
                                 scale=1.0)

            def zero_fill(dst_tile, nrep):
                nc.vector.tensor_copy(
                    _strided(dst_tile[:, :], [[512, nrep], [1, 512]]),
                    _strided(warm_t[:, :], [[0, nrep], [1, 512]]))

            zero_fill(zoo8, 4)
            zero_fill(zooT, 4)
            zero_fill(zoo2, 16)

            # PE warm-up: keep the HAM clock-gate open until real work lands
            for w in range(N_WARM):
                nc.tensor.matmul(psumW, warm_t[:, 0:128], warm_t[:, 0:512],
                                 start=True, stop=True)

            nc.vector.tensor_copy(
                _strided(zoo8[:, :], [[136, 16], [1, 8]]),
                _strided(p8[:, :], [[0, 16], [1, 8]]))
            nc.vector.tensor_copy(
                _strided(zoo2[:, :], [[130, 64], [1, 2]]),
                _strided(p2[:, :], [[0, 64], [1, 2]]))

            # ---- teacher: E = exp(10*xt - 40) in 4 sub-chunks ----
            for j in range(4):
                sl = slice(2048 * j, 2048 * (j + 1))
                nc.scalar.activation(
                    out=E[:, sl], in_=xt_t[:, sl], func=Exp, bias=bias_t,
                    scale=10.0, accum_out=OUT[:, O_ZT + j:O_ZT + j + 1])

            # ---- trace colsums on PE (ready before teacher weights) ----
            for c in range(16):
                nc.tensor.matmul(psumR, zoo8[:, 128 * c:128 * (c + 1)],
                                 xst_t[:, 512 * c:512 * (c + 1)],
                                 start=(c == 0), stop=(c == 15))

            # ---- teacher Z -> 1/Z -> weighted selectors ----
            nc.vector.tensor_scalar(
                out=OUT[:, O_ZT:O_ZT + 4], in0=OUT[:, O_ZT:O_ZT + 4],
                scalar1=1.0, scalar2=None, op0=MUL, op1=ADD,
                accum_out=OUT[:, O_Z1:O_Z1 + 1])
            nc.tensor.matmul(psumZ, b16t, OUT[:, O_Z1:O_Z1 + 1],
                             start=True, stop=True)
            nc.vector.reciprocal(out=rb, in_=psumZ)
            nc.vector.tensor_scalar_mul(out=rexp8, in0=p8, scalar1=rb)
            nc.vector.tensor_copy(
                _strided(zooT[:, :], [[136, 16], [1, 8]]),
                _strided(rexp8[:, :], [[0, 16], [1, 8]]))
            for c in range(16):
                nc.tensor.matmul(psumT, zooT[:, 128 * c:128 * (c + 1)],
                                 E[:, 512 * c:512 * (c + 1)],
                                 start=(c == 0), stop=(c == 15))

            # ---- trace block: exp (ACT), E*s term split DVE/ACT ----
            nc.scalar.activation(out=sc_act, in_=xst_t, func=Exp, bias=bias_s,
                                 scale=10.0, accum_out=OUT[:, O_SL:O_SL + 1])
            # tensor_tensor_reduce hangs trn2 hw; mul + fused-accum instead.
            # Two halves: half A starts as soon as E[0:4096] exists.
            nc.vector.tensor_mul(sc_ttr[:, 0:4096], E[:, 0:4096],
                                 xst_t[:, 0:4096])
            nc.vector.tensor_scalar(
                out=sc_ttr[:, 0:4096], in0=sc_ttr[:, 0:4096],
                scalar1=1.0, scalar2=None, op0=MUL, op1=ADD,
                accum_out=OUT[:, O_TR:O_TR + 1])
            nc.vector.tensor_mul(sc_ttr[:, 4096:8192], E[:, 4096:8192],
                                 xst_t[:, 4096:8192])
            nc.vector.tensor_copy(OUT[:, O_CR:O_CR + 512], psumR)

            # ---- main blocks ----
            Copy = mybir.ActivationFunctionType.Copy
            for i in range(4):
                w = DVE_W[i]
                aw = FB - w
                if i < 3:
                    nc.scalar.activation(
                        out=sc_act[:, 0:aw], in_=xsm_t[i][:, 0:aw], func=Exp,
                        bias=bias_s, scale=10.0,
                        accum_out=OUT[:, O_SMA + i:O_SMA + i + 1])
                else:
                    # tail chunk: sub-split so compute tracks the sub-loads
                    for j in range(4):
                        sl = slice(2048 * j, 2048 * (j + 1))
                        nc.scalar.activation(
                            out=sc_act[:, sl], in_=xsm_t[i][:, sl], func=Exp,
                            bias=bias_s, scale=10.0,
                            accum_out=OUT[:, O_SM3 + j:O_SM3 + j + 1])
                if w > 0:
                    nc.vector.tensor_scalar(
                        out=sc_i32[:, 0:w], in0=xsm_t[i][:, aw:FB],
                        scalar1=FE_A, scalar2=FE_B, op0=MUL, op1=ADD)
                    fview = sc_i32[:, 0:w].bitcast(fp32)
                    nc.vector.tensor_scalar(
                        out=fview, in0=fview, scalar1=1.0, scalar2=None,
                        op0=MUL, op1=ADD,
                        accum_out=OUT[:, O_SMD + i:O_SMD + i + 1])
                for c in range(16):
                    k = 16 * i + c
                    nc.tensor.matmul(psumM, zoo2[:, 128 * k:128 * (k + 1)],
                                     xsm_t[i][:, 512 * c:512 * (c + 1)],
                                     start=(k == 0), stop=(k == 63))
                if i == 0:
                    nc.vector.tensor_copy(OUT[:, O_CT:O_CT + 512], psumT)
                    # second product half reduced on DVE
                    nc.vector.tensor_scalar(
                        out=sc_ttr[:, 4096:8192], in0=sc_ttr[:, 4096:8192],
                        scalar1=1.0, scalar2=None, op0=MUL, op1=ADD,
                        accum_out=OUT[:, O_TR2:O_TR2 + 1])
            nc.vector.tensor_copy(OUT[:, O_CM:O_CM + 512], psumM)

            # ---- write out (colsum_P/trace early, the rest at the end) ----
            nc.sync.dma_start(out=o_out[:, 0:1024], in_=OUT[:, 0:1024])
            nc.sync.dma_start(out=o_out[:, 1024:O_W], in_=OUT[:, 1024:O_W])

    nc.compile()
    return nc


def _get_program():
    key = ("nc_v2", DVE_W)
    if key not in _CACHE:
        _CACHE[key] = _build_program()
    return _CACHE[key]


def _host_tensors():
    par8 = np.tile(np.eye(8, dtype=np.float32), (16, 1))
    par2 = np.tile(np.eye(2, dtype=np.float32), (64, 1))
    b16 = np.kron(np.eye(16, dtype=np.float32), np.ones((8, 8), np.float32))
    return par8, par2, b16


def _make_in_maps(student_output, teacher_output, center, epoch):
    import ml_dtypes

    bf = ml_dtypes.bfloat16
    S = np.asarray(student_output, dtype=np.float32).reshape(-1, D)   # [640, D]
    T = np.asarray(teacher_output, dtype=np.float32).reshape(-1, D)   # [128, D]
    cen = np.asarray(center, dtype=np.float32).reshape(1, D)
    ep = int(np.asarray(epoch))
    if ep < 30:
        t_temp = 0.04 + (0.07 - 0.04) * ep / 30
    else:
        t_temp = 0.07

    # fold center + temperature so the device uses one scale (10)
    tpre = ((T - cen) * np.float32(1.0 / (t_temp * 10.0))).astype(bf)
    Sb = S.astype(bf)

    par8, par2, b16 = _host_tensors()
    par8 = par8.astype(bf)
    par2 = par2.astype(bf)
    in_maps = []
    for k in range(NCORES):
        in_maps.append({
            "xt": np.ascontiguousarray(
                tpre[16 * k:16 * (k + 1)].reshape(128, FB)),
            "xst": np.ascontiguousarray(
                Sb[16 * k:16 * (k + 1)].reshape(128, FB)),
            "xsm": np.ascontiguousarray(
                Sb[128 + 64 * k:128 + 64 * (k + 1)].reshape(128, 4 * FB)),
            "par8": par8, "par2": par2, "b16": b16,
        })
    return in_maps


def _combine(outs):
    """outs: list of 8 [128, O_W] f32 arrays -> final loss (f64 host math)."""
    colP = np.zeros(D)
    colS = np.zeros(D)
    C = 0.0
    C128 = 0.0
    TR = 0.0
    for k in range(NCORES):
        o = outs[k].astype(np.float64)
        # teacher colsum_P: [8c+e, j] -> d = e*8192 + c*512 + j
        colP += o[:, O_CT:O_CT + 512].reshape(16, 8, 512) \
            .transpose(1, 0, 2).reshape(-1)
        colS += o[:, O_CR:O_CR + 512].reshape(16, 8, 512) \
            .transpose(1, 0, 2).reshape(-1)
        # main colsum: [2k+s, j] -> d = s*32768 + k*512 + j
        colS += o[:, O_CM:O_CM + 512].reshape(64, 2, 512) \
            .transpose(1, 0, 2).reshape(-1)

        z = o[:, O_ZT:O_ZT + 4].sum(axis=1).reshape(16, 8).sum(axis=1)
        zs_tr = o[:, O_SL].reshape(16, 8).sum(axis=1)
        c_tr = KS + np.log(zs_tr)
        zs_m = o[:, O_SMA:O_SMA + 4].copy()
        for i in range(4):
            if DVE_W[i] > 0:
                zs_m[:, i] += o[:, O_SMD + i] / FE_CORR
            if DVE_W[i] == FB:
                zs_m[:, i] = o[:, O_SMD + i] / FE_CORR
        zs_rows = (zs_m.sum(axis=1) + o[:, O_SM3:O_SM3 + 4].sum(axis=1)) \
            .reshape(64, 2).sum(axis=1)
        c_m = KS + np.log(zs_rows)
        tr_rows = (o[:, O_TR] + o[:, O_TR2]).reshape(16, 8).sum(axis=1)

        C += c_tr.sum() + c_m.sum()
        C128 += c_tr.sum()
        TR += (10.0 * tr_rows / z).sum()

    s_pl = colP @ (10.0 * colS)
    total = s_pl - 128.0 * C - TR + C128
    loss = -total / (128.0 * 639.0)
    return np.array(loss, dtype=np.float32)


def kernel(student_output, teacher_output, center, epoch):
    from concourse.bass_utils import run_bass_kernel_spmd

    global LAST_RESULTS

    in_maps = _make_in_maps(student_output, teacher_output, center, epoch)
    nc = _get_program()
    res = run_bass_kernel_spmd(
        nc, in_maps, core_ids=list(range(NCORES)), trace=TRACE)
    LAST_RESULTS = res
    return _combine([res.results[k]["out"] for k in range(NCORES)])



# revision 10
# speedup vs baseline: 1.0744x; 1.0527x over previous
"""DINO loss kernel for Trainium2 (8 NeuronCores, Bass/Tile) — v2.

Math: with S = student.reshape(640, D), T = teacher.reshape(128, D),
P = softmax((T - center)/tau), L = log_softmax(S/0.1), M = P @ L.T,
loss = -(sum(M) - trace(M)) / (128*639).

Decomposition (s = 10*S, c_v = logsumexp_d(s[v])):
  sum(M)   = dot(colsum_P, colsum_s) - 128*C        C   = sum_v c_v
  trace(M) = TR - C128                              TR  = sum_i dot(P_i, s_i)

Row sharding: core k owns teacher rows [16k,16k+16), student rows
[16k,16k+16) ("trace" block) and [128+64k, 128+64k+64) ("main" block).
Teacher/trace blocks pack into 128 partitions as (row, 8 segs of 8192);
main packs as (row, 2 segs of 32768). All inputs are bf16 (tolerance is
2e-2; bf16 costs ~1e-4).

Per-core engine assignment:
  ACT    exp of teacher (shift 40) + trace + most main columns (shift 15),
         row-sum partials via fused accum_out (fp32).
  DVE    fast-exp (2^x bit trick: y*A+B -> int32 -> bitcast f32) on a slice
         of the main columns, tensor_tensor_reduce for the trace term,
         PSUM evictions, zoo builds. Fast-exp sums carry a known +4.07%
         chord bias, corrected on the host.
  PE     column sums as matmuls with stationary zero-padded [128,128]
         selector weights, 64 (or 16) matmuls accumulating into one
         [128,512] PSUM bank. Teacher selectors carry 1/Z so the softmax
         scaling is fused into the reduction.
Scalars/column-sums are combined on the host in f64.

Measured anatomy at ~57-59us exec (trace-verified): ~7us NEFF preamble,
~37us bf16 input stream (near HBM peak, fully overlapped), ACT exp chain
ends ~51us (critical path; 1 elem/lane/cycle @1.2GHz is a hard floor),
DVE co-finishes ~50us, then a ~7us tail: ~4us HWDGE issue+descriptor
latency for the final output DMA, ~4us HBM-write completion-semaphore
latency (doc'd WAW delay), ~2.5us postamble barriers. Next levers (each
~2-3us, at jitter level): per-chunk psumM banks (M0..3, reuse psumT for
warm-ups) to evict/flush stageM incrementally; fp8 for exp-only main
columns. Do NOT: gpsimd elementwise (starves DVE via shared SBUF port),
tensor_tensor_reduce (hangs HW), gpsimd tensor_scalar (ncc rejects),
untagged pool tiles (alias one slot), in-place accum ops (drop to 1x).
"""

import dataclasses
import numpy as np

D = 65536
NCORES = 8
FB = 8192                  # big free-dim chunk (per-partition columns)
KT = 40.0                  # teacher exp shift: exp(t - KT), cancels in softmax
KS = 15.0                  # student exp shift: logsumexp = KS + log(sum exp(s-KS))

LOG2E = float(np.log2(np.e))
FE_A = 10.0 * LOG2E * (1 << 23)            # fast-exp scale (s = 10*x)
FE_B = (127.0 - KS * LOG2E) * (1 << 23)    # fast-exp bias
FE_CORR = 1.0406620487668925               # E[(1+f)/2^f], f~U[0,1): chord bias

# Per main chunk i (4 chunks of 8192): DVE fast-exp takes the last DVE_W[i]
# columns, ACT the rest.
DVE_W = (3584, 3584, 3584, 0)
N_WARM = 32                # PE warm-up matmuls (HAM clock-gate)

_CACHE = {}

TRACE = False              # test harness sets kernel.TRACE = True for profiling
LAST_RESULTS = None        # stashed BassKernelResults for the test harness

# OUT tile column map (fp32 [128, 1552])
O_CT = 0                   # 0:512    teacher colsum_P partials
O_CR = 512                 # 512:1024 trace raw colsum partials
O_CM = 1024                # 1024:1536 main raw colsum partials
O_ZT = 1536                # 1536:1540 teacher row-sum (Z) partials (4 sub)
O_SL = 1540                # trace logsumexp partials
O_TR = 1541                # trace-term partials sum_d E*s (DVE half)
O_TR2 = 1542               # trace-term partials (ACT half)
O_SMA = 1543               # 1543:1547 main ACT logsumexp partials (chunk i)
O_SMD = 1547               # 1547:1551 main DVE fast-exp partials (chunk i)
O_Z1 = 1551                # teacher Z partial total (sum of the 4 sub cols)
O_SM3 = 1552               # 1552:1556 tail-chunk sub-accums (4 x 2048 cols)
O_W = 1556


def _strided(ap, dims):
    """Replace the free dims of a 2D AP with explicit [step, num] dims."""
    return dataclasses.replace(ap, ap=[ap.ap[0]] + [list(d) for d in dims])


def _build_program():
    import concourse.tile as tile
    from concourse import bacc
    from concourse import mybir

    fp32 = mybir.dt.float32
    bf16 = mybir.dt.bfloat16
    i32 = mybir.dt.int32
    Exp = mybir.ActivationFunctionType.Exp
    AX = mybir.AxisListType.X
    MUL = mybir.AluOpType.mult
    ADD = mybir.AluOpType.add

    nc = bacc.Bacc(None, target_bir_lowering=False)

    fp8 = mybir.dt.float8e4
    xt = nc.dram_tensor("xt", [128, FB], bf16, kind="ExternalInput")
    xst = nc.dram_tensor("xst", [128, FB], bf16, kind="ExternalInput")
    # mains split by destination engine: ACT-destined columns stream as
    # fp8 (ACT reads fp8 at full rate; colsum matmuls take fp8 rhs), the
    # DVE fast-exp columns stay bf16 (fp8 DVE ops run at 0.5x). Saves
    # 2.47MB/core of shared-HBM traffic for ~6e-4 added loss error.
    xsm8 = nc.dram_tensor("xsm8", [128, 22016], fp8, kind="ExternalInput")
    xsmb = nc.dram_tensor("xsmb", [128, 10752], bf16, kind="ExternalInput")
    par8 = nc.dram_tensor("par8", [128, 8], bf16, kind="ExternalInput")
    par2 = nc.dram_tensor("par2", [128, 2], bf16, kind="ExternalInput")
    b16 = nc.dram_tensor("b16", [128, 128], fp32, kind="ExternalInput")

    o_out = nc.dram_tensor("out", [128, O_W], fp32, kind="ExternalOutput")

    with tile.TileContext(nc) as tc:
        with (
            tc.tile_pool(name="singles", bufs=1) as singles,
            tc.tile_pool(name="psum", bufs=1, space="PSUM") as psum,
        ):
            # ---- input loads (issued up-front; HW drains them in order) ----
            # teacher + trace interleaved in 2048-col sub-loads so ACT can
            # start early; mains whole except the last, split for the tail
            xt_t = singles.tile([128, FB], bf16, tag="xt_t")
            xst_t = singles.tile([128, FB], bf16, tag="xst_t")
            p8 = singles.tile([128, 8], bf16, tag="p8")
            p2 = singles.tile([128, 2], bf16, tag="p2")
            b16t = singles.tile([128, 128], fp32, tag="b16t")
            for j in range(4):
                sl = slice(2048 * j, 2048 * (j + 1))
                nc.sync.dma_start(out=xt_t[:, sl], in_=xt[:, sl])
                nc.sync.dma_start(out=xst_t[:, sl], in_=xst[:, sl])
                if j == 0:
                    nc.sync.dma_start(out=p8, in_=par8[:, :])
                    nc.sync.dma_start(out=p2, in_=par2[:, :])
                    nc.sync.dma_start(out=b16t, in_=b16[:, :])
            xa_t = []
            xd_t = []
            for i in range(4):
                aw = FB - DVE_W[i]
                ta = singles.tile([128, aw], fp8, name=f"xa{i}",
                                  tag=f"xa{i}")
                if i < 3:
                    nc.sync.dma_start(out=ta, in_=xsm8[:, 4608 * i:][:, 0:aw])
                    td = singles.tile([128, DVE_W[i]], bf16, name=f"xd{i}",
                                      tag=f"xd{i}")
                    nc.sync.dma_start(
                        out=td, in_=xsmb[:, 3584 * i:][:, 0:DVE_W[i]])
                    xd_t.append(td)
                else:
                    for j in range(4):
                        sl = slice(2048 * j, 2048 * (j + 1))
                        nc.sync.dma_start(
                            out=ta[:, sl], in_=xsm8[:, 13824:][:, sl])
                    xd_t.append(None)
                xa_t.append(ta)

            # ---- residents / scratch ----
            OUT = singles.tile([128, O_W], fp32, tag="OUT")
            E = singles.tile([128, FB], bf16, tag="E")
            zoo2 = singles.tile([128, 64 * 128], bf16, tag="zoo2")
            zoo8 = singles.tile([128, 16 * 128], bf16, tag="zoo8")
            zooT = singles.tile([128, 16 * 128], bf16, tag="zooT")
            warm_t = singles.tile([128, 512], bf16, tag="warm_t")
            sc_act = singles.tile([128, FB], bf16, tag="sc_act")
            sc_ttr = singles.tile([128, FB], bf16, tag="sc_ttr")
            sc_i32 = singles.tile([128, 4096], i32, tag="sc_i32")
            rexp8 = singles.tile([128, 8], bf16, tag="rexp8")
            rb = singles.tile([128, 1], fp32, tag="rb")
            bias_t = singles.tile([128, 1], fp32, tag="bias_t")
            bias_s = singles.tile([128, 1], fp32, tag="bias_s")

            psumT = psum.tile([128, 512], fp32, tag="psumT")
            psumR = psum.tile([128, 512], fp32, tag="psumR")
            psumM = psum.tile([128, 512], fp32, tag="psumM")
            psumW = psum.tile([128, 512], fp32, tag="psumW")
            psumZt = psum.tile([128, 512], fp32, tag="psumZt")  # full bank
            psumZ = psumZt[:, 0:1]

            # ---- constant setup (DVE memsets tiny; zoo zeroing on GPSIMD) ----
            nc.vector.memset(bias_t, -KT)
            nc.vector.memset(bias_s, -KS)
            nc.vector.memset(warm_t, 0.0)
            nc.vector.memset(OUT[:, O_ZT:O_W], 0.0)
            # prefetch the exp table set while DMA streams in
            nc.scalar.activation(out=rb, in_=bias_s, func=Exp, bias=bias_s,
                                 scale=1.0)

            def zero_fill(dst_tile, nrep):
                nc.vector.tensor_copy(
                    _strided(dst_tile[:, :], [[512, nrep], [1, 512]]),
                    _strided(warm_t[:, :], [[0, nrep], [1, 512]]))

            zero_fill(zoo8, 4)
            zero_fill(zooT, 4)
            zero_fill(zoo2, 16)

            # PE warm-up: keep the HAM clock-gate open until real work lands
            for w in range(N_WARM):
                nc.tensor.matmul(psumW, warm_t[:, 0:128], warm_t[:, 0:512],
                                 start=True, stop=True)

            nc.vector.tensor_copy(
                _strided(zoo8[:, :], [[136, 16], [1, 8]]),
                _strided(p8[:, :], [[0, 16], [1, 8]]))
            nc.vector.tensor_copy(
                _strided(zoo2[:, :], [[130, 64], [1, 2]]),
                _strided(p2[:, :], [[0, 64], [1, 2]]))

            # ---- teacher: E = exp(10*xt - 40) in 4 sub-chunks ----
            for j in range(4):
                sl = slice(2048 * j, 2048 * (j + 1))
                nc.scalar.activation(
                    out=E[:, sl], in_=xt_t[:, sl], func=Exp, bias=bias_t,
                    scale=10.0, accum_out=OUT[:, O_ZT + j:O_ZT + j + 1])

            # ---- trace colsums on PE (ready before teacher weights) ----
            for c in range(16):
                nc.tensor.matmul(psumR, zoo8[:, 128 * c:128 * (c + 1)],
                                 xst_t[:, 512 * c:512 * (c + 1)],
                                 start=(c == 0), stop=(c == 15))

            # ---- teacher Z -> 1/Z -> weighted selectors ----
            nc.vector.tensor_scalar(
                out=OUT[:, O_ZT:O_ZT + 4], in0=OUT[:, O_ZT:O_ZT + 4],
                scalar1=1.0, scalar2=None, op0=MUL, op1=ADD,
                accum_out=OUT[:, O_Z1:O_Z1 + 1])
            nc.tensor.matmul(psumZ, b16t, OUT[:, O_Z1:O_Z1 + 1],
                             start=True, stop=True)
            nc.vector.reciprocal(out=rb, in_=psumZ)
            nc.vector.tensor_scalar_mul(out=rexp8, in0=p8, scalar1=rb)
            nc.vector.tensor_copy(
                _strided(zooT[:, :], [[136, 16], [1, 8]]),
                _strided(rexp8[:, :], [[0, 16], [1, 8]]))
            for c in range(16):
                nc.tensor.matmul(psumT, zooT[:, 128 * c:128 * (c + 1)],
                                 E[:, 512 * c:512 * (c + 1)],
                                 start=(c == 0), stop=(c == 15))

            # ---- trace block: exp (ACT), E*s term split DVE/ACT ----
            nc.scalar.activation(out=sc_act, in_=xst_t, func=Exp, bias=bias_s,
                                 scale=10.0, accum_out=OUT[:, O_SL:O_SL + 1])
            # tensor_tensor_reduce hangs trn2 hw; mul + fused-accum instead.
            # Two halves: half A starts as soon as E[0:4096] exists.
            nc.vector.tensor_mul(sc_ttr[:, 0:4096], E[:, 0:4096],
                                 xst_t[:, 0:4096])
            nc.vector.tensor_scalar(
                out=sc_ttr[:, 0:4096], in0=sc_ttr[:, 0:4096],
                scalar1=1.0, scalar2=None, op0=MUL, op1=ADD,
                accum_out=OUT[:, O_TR:O_TR + 1])
            nc.vector.tensor_mul(sc_ttr[:, 4096:8192], E[:, 4096:8192],
                                 xst_t[:, 4096:8192])
            nc.vector.tensor_copy(OUT[:, O_CR:O_CR + 512], psumR)

            # ---- main blocks ----
            Copy = mybir.ActivationFunctionType.Copy
            for i in range(4):
                w = DVE_W[i]
                aw = FB - w
                if i < 3:
                    nc.scalar.activation(
                        out=sc_act[:, 0:aw], in_=xa_t[i][:, 0:aw], func=Exp,
                        bias=bias_s, scale=10.0,
                        accum_out=OUT[:, O_SMA + i:O_SMA + i + 1])
                else:
                    # tail chunk: sub-split so compute tracks the sub-loads
                    for j in range(4):
                        sl = slice(2048 * j, 2048 * (j + 1))
                        nc.scalar.activation(
                            out=sc_act[:, sl], in_=xa_t[i][:, sl], func=Exp,
                            bias=bias_s, scale=10.0,
                            accum_out=OUT[:, O_SM3 + j:O_SM3 + j + 1])
                if w > 0:
                    nc.vector.tensor_scalar(
                        out=sc_i32[:, 0:w], in0=xd_t[i][:, 0:w],
                        scalar1=FE_A, scalar2=FE_B, op0=MUL, op1=ADD)
                    fview = sc_i32[:, 0:w].bitcast(fp32)
                    nc.vector.tensor_scalar(
                        out=fview, in0=fview, scalar1=1.0, scalar2=None,
                        op0=MUL, op1=ADD,
                        accum_out=OUT[:, O_SMD + i:O_SMD + i + 1])
                na = aw // 512
                for c in range(16):
                    k = 16 * i + c
                    rhs = (xa_t[i][:, 512 * c:512 * (c + 1)] if c < na else
                           xd_t[i][:, 512 * (c - na):512 * (c - na + 1)])
                    nc.tensor.matmul(psumM, zoo2[:, 128 * k:128 * (k + 1)],
                                     rhs, start=(k == 0), stop=(k == 63))
                if i == 0:
                    nc.vector.tensor_copy(OUT[:, O_CT:O_CT + 512], psumT)
                    # second product half reduced on DVE
                    nc.vector.tensor_scalar(
                        out=sc_ttr[:, 4096:8192], in0=sc_ttr[:, 4096:8192],
                        scalar1=1.0, scalar2=None, op0=MUL, op1=ADD,
                        accum_out=OUT[:, O_TR2:O_TR2 + 1])
            nc.vector.tensor_copy(OUT[:, O_CM:O_CM + 512], psumM)

            # ---- write out (colsum_P/trace early, the rest at the end) ----
            nc.sync.dma_start(out=o_out[:, 0:1024], in_=OUT[:, 0:1024])
            nc.sync.dma_start(out=o_out[:, 1024:O_W], in_=OUT[:, 1024:O_W])

    nc.compile()
    return nc


def _get_program():
    key = ("nc_v2", DVE_W)
    if key not in _CACHE:
        _CACHE[key] = _build_program()
    return _CACHE[key]


def _host_tensors():
    par8 = np.tile(np.eye(8, dtype=np.float32), (16, 1))
    par2 = np.tile(np.eye(2, dtype=np.float32), (64, 1))
    b16 = np.kron(np.eye(16, dtype=np.float32), np.ones((8, 8), np.float32))
    return par8, par2, b16


def _make_in_maps(student_output, teacher_output, center, epoch):
    import ml_dtypes

    bf = ml_dtypes.bfloat16
    f8 = ml_dtypes.float8_e4m3fn
    S = np.asarray(student_output, dtype=np.float32).reshape(-1, D)   # [640, D]
    T = np.asarray(teacher_output, dtype=np.float32).reshape(-1, D)   # [128, D]
    cen = np.asarray(center, dtype=np.float32).reshape(1, D)
    ep = int(np.asarray(epoch))
    if ep < 30:
        t_temp = 0.04 + (0.07 - 0.04) * ep / 30
    else:
        t_temp = 0.07

    # fold center + temperature so the device uses one scale (10)
    tpre = ((T - cen) * np.float32(1.0 / (t_temp * 10.0))).astype(bf)
    Sb = S.astype(bf)

    par8, par2, b16 = _host_tensors()
    par8 = par8.astype(bf)
    par2 = par2.astype(bf)
    in_maps = []
    for k in range(NCORES):
        pk = np.ascontiguousarray(
            S[128 + 64 * k:128 + 64 * (k + 1)].reshape(128, 4 * FB))
        # mains: ACT-destined columns in fp8, DVE fast-exp columns in bf16
        xsm8 = np.concatenate(
            [pk[:, FB * i:FB * i + FB - DVE_W[i]] for i in range(4)],
            axis=1).astype(f8)
        xsmb = np.concatenate(
            [pk[:, FB * i + FB - DVE_W[i]:FB * (i + 1)] for i in range(3)],
            axis=1).astype(bf)
        in_maps.append({
            "xt": np.ascontiguousarray(
                tpre[16 * k:16 * (k + 1)].reshape(128, FB)),
            "xst": np.ascontiguousarray(
                Sb[16 * k:16 * (k + 1)].reshape(128, FB)),
            "xsm8": xsm8, "xsmb": xsmb,
            "par8": par8, "par2": par2, "b16": b16,
        })
    return in_maps


def _combine(outs):
    """outs: list of 8 [128, O_W] f32 arrays -> final loss (f64 host math)."""
    colP = np.zeros(D)
    colS = np.zeros(D)
    C = 0.0
    C128 = 0.0
    TR = 0.0
    for k in range(NCORES):
        o = outs[k].astype(np.float64)
        # teacher colsum_P: [8c+e, j] -> d = e*8192 + c*512 + j
        colP += o[:, O_CT:O_CT + 512].reshape(16, 8, 512) \
            .transpose(1, 0, 2).reshape(-1)
        colS += o[:, O_CR:O_CR + 512].reshape(16, 8, 512) \
            .transpose(1, 0, 2).reshape(-1)
        # main colsum: [2k+s, j] -> d = s*32768 + k*512 + j
        colS += o[:, O_CM:O_CM + 512].reshape(64, 2, 512) \
            .transpose(1, 0, 2).reshape(-1)

        z = o[:, O_ZT:O_ZT + 4].sum(axis=1).reshape(16, 8).sum(axis=1)
        zs_tr = o[:, O_SL].reshape(16, 8).sum(axis=1)
        c_tr = KS + np.log(zs_tr)
        zs_m = o[:, O_SMA:O_SMA + 4].copy()
        for i in range(4):
            if DVE_W[i] > 0:
                zs_m[:, i] += o[:, O_SMD + i] / FE_CORR
            if DVE_W[i] == FB:
                zs_m[:, i] = o[:, O_SMD + i] / FE_CORR
        zs_rows = (zs_m.sum(axis=1) + o[:, O_SM3:O_SM3 + 4].sum(axis=1)) \
            .reshape(64, 2).sum(axis=1)
        c_m = KS + np.log(zs_rows)
        tr_rows = (o[:, O_TR] + o[:, O_TR2]).reshape(16, 8).sum(axis=1)

        C += c_tr.sum() + c_m.sum()
        C128 += c_tr.sum()
        TR += (10.0 * tr_rows / z).sum()

    s_pl = colP @ (10.0 * colS)
    total = s_pl - 128.0 * C - TR + C128
    loss = -total / (128.0 * 639.0)
    return np.array(loss, dtype=np.float32)


def kernel(student_output, teacher_output, center, epoch):
    from concourse.bass_utils import run_bass_kernel_spmd

    global LAST_RESULTS

    in_maps = _make_in_maps(student_output, teacher_output, center, epoch)
    nc = _get_program()
    res = run_bass_kernel_spmd(
        nc, in_maps, core_ids=list(range(NCORES)), trace=TRACE)
    LAST_RESULTS = res
    return _combine([res.results[k]["out"] for k in range(NCORES)])



# revision 13
# speedup vs baseline: 1.0990x; 1.0229x over previous
"""DINO loss kernel for Trainium2 (8 NeuronCores, Bass/Tile) — v2.

Math: with S = student.reshape(640, D), T = teacher.reshape(128, D),
P = softmax((T - center)/tau), L = log_softmax(S/0.1), M = P @ L.T,
loss = -(sum(M) - trace(M)) / (128*639).

Decomposition (s = 10*S, c_v = logsumexp_d(s[v])):
  sum(M)   = dot(colsum_P, colsum_s) - 128*C        C   = sum_v c_v
  trace(M) = TR - C128                              TR  = sum_i dot(P_i, s_i)

Row sharding: core k owns teacher rows [16k,16k+16), student rows
[16k,16k+16) ("trace" block) and [128+64k, 128+64k+64) ("main" block).
Teacher/trace blocks pack into 128 partitions as (row, 8 segs of 8192);
main packs as (row, 2 segs of 32768). All inputs are bf16 (tolerance is
2e-2; bf16 costs ~1e-4).

Per-core engine assignment:
  ACT    exp of teacher (shift 40) + trace + most main columns (shift 15),
         row-sum partials via fused accum_out (fp32).
  DVE    fast-exp (2^x bit trick: y*A+B -> int32 -> bitcast f32) on a slice
         of the main columns, tensor_tensor_reduce for the trace term,
         PSUM evictions, zoo builds. Fast-exp sums carry a known +4.07%
         chord bias, corrected on the host.
  PE     column sums as matmuls with stationary zero-padded [128,128]
         selector weights, 64 (or 16) matmuls accumulating into one
         [128,512] PSUM bank. Teacher selectors carry 1/Z so the softmax
         scaling is fused into the reduction.
Scalars/column-sums are combined on the host in f64.

Measured anatomy at ~57-59us exec (trace-verified): ~7us NEFF preamble,
~37us bf16 input stream (near HBM peak, fully overlapped), ACT exp chain
ends ~51us (critical path; 1 elem/lane/cycle @1.2GHz is a hard floor),
DVE co-finishes ~50us, then a ~7us tail: ~4us HWDGE issue+descriptor
latency for the final output DMA, ~4us HBM-write completion-semaphore
latency (doc'd WAW delay), ~2.5us postamble barriers. Next levers (each
~2-3us, at jitter level): per-chunk psumM banks (M0..3, reuse psumT for
warm-ups) to evict/flush stageM incrementally; fp8 for exp-only main
columns. Do NOT: gpsimd elementwise (starves DVE via shared SBUF port),
tensor_tensor_reduce (hangs HW), gpsimd tensor_scalar (ncc rejects),
untagged pool tiles (alias one slot), in-place accum ops (drop to 1x).
"""

import dataclasses
import numpy as np

D = 65536
NCORES = 8
FB = 8192                  # big free-dim chunk (per-partition columns)
KT = 40.0                  # teacher exp shift: exp(t - KT), cancels in softmax
KS = 15.0                  # student exp shift: logsumexp = KS + log(sum exp(s-KS))

LOG2E = float(np.log2(np.e))
FE_A = 10.0 * LOG2E * (1 << 23)            # fast-exp scale (s = 10*x)
FE_B = (127.0 - KS * LOG2E) * (1 << 23)    # fast-exp bias
FE_CORR = 1.0406620487668925               # E[(1+f)/2^f], f~U[0,1): chord bias

# Per main chunk i (4 chunks of 8192): DVE fast-exp takes the last DVE_W[i]
# columns, ACT the rest.
DVE_W = (3584, 3584, 3584, 0)
N_WARM = 32                # PE warm-up matmuls (HAM clock-gate)

_CACHE = {}

TRACE = False              # test harness sets kernel.TRACE = True for profiling
LAST_RESULTS = None        # stashed BassKernelResults for the test harness

# OUT tile column map (fp32 [128, 1552])
O_CT = 0                   # 0:512    teacher colsum_P partials
O_CR = 512                 # 512:1024 trace raw colsum partials
O_CM = 1024                # 1024:1536 main raw colsum partials
O_ZT = 1536                # 1536:1540 teacher row-sum (Z) partials (4 sub)
O_SL = 1540                # trace logsumexp partials
O_TR = 1541                # trace-term partials sum_d E*s (DVE half)
O_TR2 = 1542               # trace-term partials (ACT half)
O_SMA = 1543               # 1543:1547 main ACT logsumexp partials (chunk i)
O_SMD = 1547               # 1547:1551 main DVE fast-exp partials (chunk i)
O_Z1 = 1551                # teacher Z partial total (sum of the 4 sub cols)
O_SM3 = 1552               # 1552:1556 tail-chunk sub-accums (4 x 2048 cols)
O_W = 1556


def _strided(ap, dims):
    """Replace the free dims of a 2D AP with explicit [step, num] dims."""
    return dataclasses.replace(ap, ap=[ap.ap[0]] + [list(d) for d in dims])


def _build_program():
    import concourse.tile as tile
    from concourse import bacc
    from concourse import mybir

    fp32 = mybir.dt.float32
    bf16 = mybir.dt.bfloat16
    i32 = mybir.dt.int32
    Exp = mybir.ActivationFunctionType.Exp
    AX = mybir.AxisListType.X
    MUL = mybir.AluOpType.mult
    ADD = mybir.AluOpType.add

    nc = bacc.Bacc(None, target_bir_lowering=False)

    fp8 = mybir.dt.float8e4
    # teacher streams as fp8 too: it only feeds ACT exp, and quantized
    # logits renormalize inside the softmax P (another 1MB/core saved)
    xt = nc.dram_tensor("xt", [128, FB], fp8, kind="ExternalInput")
    xst = nc.dram_tensor("xst", [128, FB], bf16, kind="ExternalInput")
    # mains split by destination engine: ACT-destined columns stream as
    # fp8 (ACT reads fp8 at full rate; colsum matmuls take fp8 rhs), the
    # DVE fast-exp columns stay bf16 (fp8 DVE ops run at 0.5x). Saves
    # 2.47MB/core of shared-HBM traffic for ~6e-4 added loss error.
    xsm8 = nc.dram_tensor("xsm8", [128, 22016], fp8, kind="ExternalInput")
    xsmb = nc.dram_tensor("xsmb", [128, 10752], bf16, kind="ExternalInput")
    par8 = nc.dram_tensor("par8", [128, 8], bf16, kind="ExternalInput")
    par2 = nc.dram_tensor("par2", [128, 2], bf16, kind="ExternalInput")
    b16 = nc.dram_tensor("b16", [128, 128], fp32, kind="ExternalInput")

    o_out = nc.dram_tensor("out", [128, O_W], fp32, kind="ExternalOutput")

    with tile.TileContext(nc) as tc:
        with (
            tc.tile_pool(name="singles", bufs=1) as singles,
            tc.tile_pool(name="psum", bufs=1, space="PSUM") as psum,
        ):
            # ---- input loads (issued up-front; HW drains them in order) ----
            # teacher + trace interleaved in 2048-col sub-loads so ACT can
            # start early; mains whole except the last, split for the tail
            xt_t = singles.tile([128, FB], fp8, tag="xt_t")
            xst_t = singles.tile([128, FB], bf16, tag="xst_t")
            p8 = singles.tile([128, 8], bf16, tag="p8")
            p2 = singles.tile([128, 2], bf16, tag="p2")
            b16t = singles.tile([128, 128], fp32, tag="b16t")
            for j in range(4):
                sl = slice(2048 * j, 2048 * (j + 1))
                nc.sync.dma_start(out=xt_t[:, sl], in_=xt[:, sl])
                nc.sync.dma_start(out=xst_t[:, sl], in_=xst[:, sl])
                if j == 0:
                    nc.sync.dma_start(out=p8, in_=par8[:, :])
                    nc.sync.dma_start(out=p2, in_=par2[:, :])
                    nc.sync.dma_start(out=b16t, in_=b16[:, :])
            xa_t = []
            xd_t = []
            for i in range(4):
                aw = FB - DVE_W[i]
                ta = singles.tile([128, aw], fp8, name=f"xa{i}",
                                  tag=f"xa{i}")
                if i < 3:
                    nc.sync.dma_start(out=ta, in_=xsm8[:, 4608 * i:][:, 0:aw])
                    td = singles.tile([128, DVE_W[i]], bf16, name=f"xd{i}",
                                      tag=f"xd{i}")
                    nc.sync.dma_start(
                        out=td, in_=xsmb[:, 3584 * i:][:, 0:DVE_W[i]])
                    xd_t.append(td)
                else:
                    for j in range(4):
                        sl = slice(2048 * j, 2048 * (j + 1))
                        nc.sync.dma_start(
                            out=ta[:, sl], in_=xsm8[:, 13824:][:, sl])
                    xd_t.append(None)
                xa_t.append(ta)

            # ---- residents / scratch ----
            OUT = singles.tile([128, O_W], fp32, tag="OUT")
            E = singles.tile([128, FB], bf16, tag="E")
            zoo2 = singles.tile([128, 64 * 128], bf16, tag="zoo2")
            zoo8 = singles.tile([128, 16 * 128], bf16, tag="zoo8")
            zooT = singles.tile([128, 16 * 128], bf16, tag="zooT")
            warm_t = singles.tile([128, 512], bf16, tag="warm_t")
            sc_act = singles.tile([128, FB], bf16, tag="sc_act")
            sc_ttr = singles.tile([128, FB], bf16, tag="sc_ttr")
            sc_i32 = singles.tile([128, 4096], i32, tag="sc_i32")
            rexp8 = singles.tile([128, 8], bf16, tag="rexp8")
            rb = singles.tile([128, 1], fp32, tag="rb")
            bias_t = singles.tile([128, 1], fp32, tag="bias_t")
            bias_s = singles.tile([128, 1], fp32, tag="bias_s")

            psumT = psum.tile([128, 512], fp32, tag="psumT")
            psumR = psum.tile([128, 512], fp32, tag="psumR")
            psumM = psum.tile([128, 512], fp32, tag="psumM")
            psumW = psum.tile([128, 512], fp32, tag="psumW")
            psumZt = psum.tile([128, 512], fp32, tag="psumZt")  # full bank
            psumZ = psumZt[:, 0:1]

            # ---- constant setup (DVE memsets tiny; zoo zeroing on GPSIMD) ----
            nc.vector.memset(bias_t, -KT)
            nc.vector.memset(bias_s, -KS)
            nc.vector.memset(warm_t, 0.0)
            nc.vector.memset(OUT[:, O_ZT:O_W], 0.0)
            # prefetch the exp table set while DMA streams in
            nc.scalar.activation(out=rb, in_=bias_s, func=Exp, bias=bias_s,
                                 scale=1.0)

            def zero_fill(dst_tile, nrep):
                nc.vector.tensor_copy(
                    _strided(dst_tile[:, :], [[512, nrep], [1, 512]]),
                    _strided(warm_t[:, :], [[0, nrep], [1, 512]]))

            zero_fill(zoo8, 4)
            zero_fill(zooT, 4)
            zero_fill(zoo2, 16)

            # PE warm-up: keep the HAM clock-gate open until real work lands
            for w in range(N_WARM):
                nc.tensor.matmul(psumW, warm_t[:, 0:128], warm_t[:, 0:512],
                                 start=True, stop=True)

            nc.vector.tensor_copy(
                _strided(zoo8[:, :], [[136, 16], [1, 8]]),
                _strided(p8[:, :], [[0, 16], [1, 8]]))
            nc.vector.tensor_copy(
                _strided(zoo2[:, :], [[130, 64], [1, 2]]),
                _strided(p2[:, :], [[0, 64], [1, 2]]))

            # ---- teacher: E = exp(10*xt - 40) in 4 sub-chunks ----
            for j in range(4):
                sl = slice(2048 * j, 2048 * (j + 1))
                nc.scalar.activation(
                    out=E[:, sl], in_=xt_t[:, sl], func=Exp, bias=bias_t,
                    scale=10.0, accum_out=OUT[:, O_ZT + j:O_ZT + j + 1])

            # ---- trace colsums on PE (ready before teacher weights) ----
            for c in range(16):
                nc.tensor.matmul(psumR, zoo8[:, 128 * c:128 * (c + 1)],
                                 xst_t[:, 512 * c:512 * (c + 1)],
                                 start=(c == 0), stop=(c == 15))

            # ---- teacher Z -> 1/Z -> weighted selectors ----
            nc.vector.tensor_scalar(
                out=OUT[:, O_ZT:O_ZT + 4], in0=OUT[:, O_ZT:O_ZT + 4],
                scalar1=1.0, scalar2=None, op0=MUL, op1=ADD,
                accum_out=OUT[:, O_Z1:O_Z1 + 1])
            nc.tensor.matmul(psumZ, b16t, OUT[:, O_Z1:O_Z1 + 1],
                             start=True, stop=True)
            nc.vector.reciprocal(out=rb, in_=psumZ)
            nc.vector.tensor_scalar_mul(out=rexp8, in0=p8, scalar1=rb)
            nc.vector.tensor_copy(
                _strided(zooT[:, :], [[136, 16], [1, 8]]),
                _strided(rexp8[:, :], [[0, 16], [1, 8]]))
            for c in range(16):
                nc.tensor.matmul(psumT, zooT[:, 128 * c:128 * (c + 1)],
                                 E[:, 512 * c:512 * (c + 1)],
                                 start=(c == 0), stop=(c == 15))

            # ---- trace block: exp (ACT), E*s term split DVE/ACT ----
            nc.scalar.activation(out=sc_act, in_=xst_t, func=Exp, bias=bias_s,
                                 scale=10.0, accum_out=OUT[:, O_SL:O_SL + 1])
            # tensor_tensor_reduce hangs trn2 hw; mul + fused-accum instead.
            # Two halves: half A starts as soon as E[0:4096] exists.
            nc.vector.tensor_mul(sc_ttr[:, 0:4096], E[:, 0:4096],
                                 xst_t[:, 0:4096])
            nc.vector.tensor_scalar(
                out=sc_ttr[:, 0:4096], in0=sc_ttr[:, 0:4096],
                scalar1=1.0, scalar2=None, op0=MUL, op1=ADD,
                accum_out=OUT[:, O_TR:O_TR + 1])
            nc.vector.tensor_mul(sc_ttr[:, 4096:8192], E[:, 4096:8192],
                                 xst_t[:, 4096:8192])
            nc.vector.tensor_copy(OUT[:, O_CR:O_CR + 512], psumR)

            # ---- main blocks ----
            Copy = mybir.ActivationFunctionType.Copy
            for i in range(4):
                w = DVE_W[i]
                aw = FB - w
                if i < 3:
                    nc.scalar.activation(
                        out=sc_act[:, 0:aw], in_=xa_t[i][:, 0:aw], func=Exp,
                        bias=bias_s, scale=10.0,
                        accum_out=OUT[:, O_SMA + i:O_SMA + i + 1])
                else:
                    # tail chunk: sub-split so compute tracks the sub-loads
                    for j in range(4):
                        sl = slice(2048 * j, 2048 * (j + 1))
                        nc.scalar.activation(
                            out=sc_act[:, sl], in_=xa_t[i][:, sl], func=Exp,
                            bias=bias_s, scale=10.0,
                            accum_out=OUT[:, O_SM3 + j:O_SM3 + j + 1])
                if w > 0:
                    nc.vector.tensor_scalar(
                        out=sc_i32[:, 0:w], in0=xd_t[i][:, 0:w],
                        scalar1=FE_A, scalar2=FE_B, op0=MUL, op1=ADD)
                    fview = sc_i32[:, 0:w].bitcast(fp32)
                    nc.vector.tensor_scalar(
                        out=fview, in0=fview, scalar1=1.0, scalar2=None,
                        op0=MUL, op1=ADD,
                        accum_out=OUT[:, O_SMD + i:O_SMD + i + 1])
                na = aw // 512
                for c in range(16):
                    k = 16 * i + c
                    rhs = (xa_t[i][:, 512 * c:512 * (c + 1)] if c < na else
                           xd_t[i][:, 512 * (c - na):512 * (c - na + 1)])
                    nc.tensor.matmul(psumM, zoo2[:, 128 * k:128 * (k + 1)],
                                     rhs, start=(k == 0), stop=(k == 63))
                if i == 0:
                    nc.vector.tensor_copy(OUT[:, O_CT:O_CT + 512], psumT)
                    # second product half reduced on DVE
                    nc.vector.tensor_scalar(
                        out=sc_ttr[:, 4096:8192], in0=sc_ttr[:, 4096:8192],
                        scalar1=1.0, scalar2=None, op0=MUL, op1=ADD,
                        accum_out=OUT[:, O_TR2:O_TR2 + 1])
            nc.vector.tensor_copy(OUT[:, O_CM:O_CM + 512], psumM)

            # ---- write out (colsum_P/trace early, the rest at the end) ----
            nc.sync.dma_start(out=o_out[:, 0:1024], in_=OUT[:, 0:1024])
            nc.sync.dma_start(out=o_out[:, 1024:O_W], in_=OUT[:, 1024:O_W])

    nc.compile()
    return nc


def _get_program():
    key = ("nc_v2", DVE_W)
    if key not in _CACHE:
        _CACHE[key] = _build_program()
    return _CACHE[key]


def _host_tensors():
    par8 = np.tile(np.eye(8, dtype=np.float32), (16, 1))
    par2 = np.tile(np.eye(2, dtype=np.float32), (64, 1))
    b16 = np.kron(np.eye(16, dtype=np.float32), np.ones((8, 8), np.float32))
    return par8, par2, b16


def _make_in_maps(student_output, teacher_output, center, epoch):
    import ml_dtypes

    bf = ml_dtypes.bfloat16
    f8 = ml_dtypes.float8_e4m3fn
    S = np.asarray(student_output, dtype=np.float32).reshape(-1, D)   # [640, D]
    T = np.asarray(teacher_output, dtype=np.float32).reshape(-1, D)   # [128, D]
    cen = np.asarray(center, dtype=np.float32).reshape(1, D)
    ep = int(np.asarray(epoch))
    if ep < 30:
        t_temp = 0.04 + (0.07 - 0.04) * ep / 30
    else:
        t_temp = 0.07

    # fold center + temperature so the device uses one scale (10)
    tpre = ((T - cen) * np.float32(1.0 / (t_temp * 10.0))).astype(f8)
    Sb = S.astype(bf)

    par8, par2, b16 = _host_tensors()
    par8 = par8.astype(bf)
    par2 = par2.astype(bf)
    in_maps = []
    for k in range(NCORES):
        pk = np.ascontiguousarray(
            S[128 + 64 * k:128 + 64 * (k + 1)].reshape(128, 4 * FB))
        # mains: ACT-destined columns in fp8, DVE fast-exp columns in bf16
        xsm8 = np.concatenate(
            [pk[:, FB * i:FB * i + FB - DVE_W[i]] for i in range(4)],
            axis=1).astype(f8)
        xsmb = np.concatenate(
            [pk[:, FB * i + FB - DVE_W[i]:FB * (i + 1)] for i in range(3)],
            axis=1).astype(bf)
        in_maps.append({
            "xt": np.ascontiguousarray(
                tpre[16 * k:16 * (k + 1)].reshape(128, FB)),
            "xst": np.ascontiguousarray(
                Sb[16 * k:16 * (k + 1)].reshape(128, FB)),
            "xsm8": xsm8, "xsmb": xsmb,
            "par8": par8, "par2": par2, "b16": b16,
        })
    return in_maps


def _combine(outs):
    """outs: list of 8 [128, O_W] f32 arrays -> final loss (f64 host math)."""
    colP = np.zeros(D)
    colS = np.zeros(D)
    C = 0.0
    C128 = 0.0
    TR = 0.0
    for k in range(NCORES):
        o = outs[k].astype(np.float64)
        # teacher colsum_P: [8c+e, j] -> d = e*8192 + c*512 + j
        colP += o[:, O_CT:O_CT + 512].reshape(16, 8, 512) \
            .transpose(1, 0, 2).reshape(-1)
        colS += o[:, O_CR:O_CR + 512].reshape(16, 8, 512) \
            .transpose(1, 0, 2).reshape(-1)
        # main colsum: [2k+s, j] -> d = s*32768 + k*512 + j
        colS += o[:, O_CM:O_CM + 512].reshape(64, 2, 512) \
            .transpose(1, 0, 2).reshape(-1)

        z = o[:, O_ZT:O_ZT + 4].sum(axis=1).reshape(16, 8).sum(axis=1)
        zs_tr = o[:, O_SL].reshape(16, 8).sum(axis=1)
        c_tr = KS + np.log(zs_tr)
        zs_m = o[:, O_SMA:O_SMA + 4].copy()
        for i in range(4):
            if DVE_W[i] > 0:
                zs_m[:, i] += o[:, O_SMD + i] / FE_CORR
            if DVE_W[i] == FB:
                zs_m[:, i] = o[:, O_SMD + i] / FE_CORR
        zs_rows = (zs_m.sum(axis=1) + o[:, O_SM3:O_SM3 + 4].sum(axis=1)) \
            .reshape(64, 2).sum(axis=1)
        c_m = KS + np.log(zs_rows)
        tr_rows = (o[:, O_TR] + o[:, O_TR2]).reshape(16, 8).sum(axis=1)

        C += c_tr.sum() + c_m.sum()
        C128 += c_tr.sum()
        TR += (10.0 * tr_rows / z).sum()

    s_pl = colP @ (10.0 * colS)
    total = s_pl - 128.0 * C - TR + C128
    loss = -total / (128.0 * 639.0)
    return np.array(loss, dtype=np.float32)


def kernel(student_output, teacher_output, center, epoch):
    from concourse.bass_utils import run_bass_kernel_spmd

    global LAST_RESULTS

    in_maps = _make_in_maps(student_output, teacher_output, center, epoch)
    nc = _get_program()
    res = run_bass_kernel_spmd(
        nc, in_maps, core_ids=list(range(NCORES)), trace=TRACE)
    LAST_RESULTS = res
    return _combine([res.results[k]["out"] for k in range(NCORES)])

